# revision 11
# baseline (speedup 1.0000x reference)
"""Trainium2 Bass kernel for nn_MultiHeadAttn (B=2, S=2048, D=1024, H=16,
ADIM=64, rel-pos bias vocab 33).

Sharding: batch x head-group over 8 cores. Core c handles batch b=c//4 and
heads [4*(c%4), 4*(c%4)+4). Each core computes q/k/v projections for its 256
model dims, attention for its 4 heads, and a partial output projection; the
host sums the 4 partials per batch.

Rel-pos bias (same trick as before): scoresT[s,t] uses k VARIANTS so the far
field is free (kLo = k + pemb[32] for s-t >= 2 tiles, kHi = k + pemb[0] for
t-s >= 2 tiles); the <=3 diagonal-crossing tiles get their bias
multiplicatively after exp via a host-precomputed band.

This version:
  * AV is swapped: v (with a ones column for the denominator) is the
    STATIONARY operand, expT the moving one -> ctxT accumulates directly in
    PSUM as [65, q] (2 matmuls of N=512 per (head, q-half, s-tile) instead
    of 16 matmuls of N=65). No PE transposes needed for the out projection.
  * q is processed in two 1024-col halves per head so ctx PSUM is 2 banks,
    leaving banks for projections to interleave into the softmax loop:
    v-projection fills head 0, the mt=1 q/k projections fill head 1, and
    the first half of the output projection fills head 3's second half.
  * Softmax normalization: reciprocal of the PSUM denominator row, a K=1
    ones-matmul broadcasts it across partitions, one vector multiply
    normalizes and casts; odd heads are packed into partitions 64..127 of
    the pair tile via a small SBUF->SBUF DMA.
  * Partial outputs returned in bf16 (halves the output DMA).
"""
import numpy as np
import ml_dtypes

import concourse.bacc as bacc
import concourse.mybir as mybir
import concourse.tile as tile
from concourse.bass_utils import run_bass_kernel_spmd

B, S, D = 2, 2048, 1024
H, ADIM, K_REL, NJ = 16, 64, 16, 33
HPC = 4            # heads per core
DHC = HPC * ADIM   # 256 model dims per core
P = 128
NST = S // P       # 16 s-tiles
NKC = D // P       # 8 contraction chunks for projections
QH = 1024          # q processed in halves
BF16 = mybir.dt.bfloat16
FP32 = mybir.dt.float32

_COMPILED = None


def build_nc():
    nc = bacc.Bacc(None, target_bir_lowering=False)
    with tile.TileContext(nc) as tc:
        x_d = {nm: nc.dram_tensor(f"x{nm}", [P, NKC * S], BF16,
                                  kind="ExternalInput") for nm in "qkv"}
        w_d = {nm: nc.dram_tensor(f"w{nm}", [P, NKC * DHC], BF16,
                                  kind="ExternalInput") for nm in "qkv"}
        wo_d = nc.dram_tensor("wo", [P, 2 * D], BF16, kind="ExternalInput")
        pemb0_d = nc.dram_tensor("pemb0", [P, 1], FP32, kind="ExternalInput")
        pemb32_d = nc.dram_tensor("pemb32", [P, 1], FP32, kind="ExternalInput")
        band_d = nc.dram_tensor("band", [HPC, P, NST * 3 * P], BF16,
                                kind="ExternalInput")
        out_d = nc.dram_tensor("out", [S, D], BF16, kind="ExternalOutput")

        from contextlib import ExitStack
        with ExitStack() as stack:
            const = stack.enter_context(tc.tile_pool(name="const", bufs=1))
            pemb0_sb = const.tile([P, 1], FP32)
            pemb32_sb = const.tile([P, 1], FP32)
            ones_sb = const.tile([65, P], BF16)
            nc.vector.memset(ones_sb[:], 1.0)
            nc.sync.dma_start(out=pemb0_sb[:], in_=pemb0_d[:])
            nc.sync.dma_start(out=pemb32_sb[:], in_=pemb32_d[:])

            persist = stack.enter_context(tc.tile_pool(name="persist", bufs=1))
            qT_sb = [persist.tile([P, S], BF16, name=f"qT{i}") for i in range(2)]
            kT_sb = [persist.tile([P, S], BF16, name=f"kT{i}") for i in range(2)]
            kLo_sb = [persist.tile([P, S], BF16, name=f"kLo{i}") for i in range(2)]
            kHi_sb = [persist.tile([P, S], BF16, name=f"kHi{i}") for i in range(2)]
            v_sb = [persist.tile([P, HPC * 65], BF16, name=f"v{st}")
                    for st in range(NST)]
            ctxT2_sb = [persist.tile([P, S], BF16, name=f"ctxT2{i}")
                        for i in range(2)]
            wo_sb = persist.tile([P, 2 * D], BF16, name="wo")
            tmp_sb = persist.tile([64, QH], BF16, name="tmp")
            dn_sb = persist.tile([65, QH], BF16, name="dn")
            bc_sb = persist.tile([64, QH], BF16, name="bc")

            xin = stack.enter_context(tc.tile_pool(name="xin", bufs=1))
            w_in = stack.enter_context(tc.tile_pool(name="w_in", bufs=1))
            x_sb = {nm: xin.tile([P, NKC * S], BF16, name=f"x{nm}")
                    for nm in "qkv"}
            w_sb = {nm: w_in.tile([P, NKC * DHC], BF16, name=f"w{nm}")
                    for nm in "qkv"}

            ppsum = stack.enter_context(
                tc.tile_pool(name="ppsum", bufs=2, space="PSUM"))
            spsum = stack.enter_context(
                tc.tile_pool(name="spsum", bufs=2, space="PSUM"))
            cpsum = stack.enter_context(
                tc.tile_pool(name="cpsum", bufs=1, space="PSUM"))
            epool = stack.enter_context(tc.tile_pool(name="expT", bufs=3))
            bpool = stack.enter_context(tc.tile_pool(name="band", bufs=2))
            ostage = stack.enter_context(tc.tile_pool(name="ostage", bufs=3))

            # ---- input DMAs: xq + xv on sync queue, xk + bands on scalar
            # queue (both are HWDGE queues; scalar is idle until first exp) ----
            nc.sync.dma_start(out=w_sb["q"][:], in_=w_d["q"][:])
            for ch in range(4):
                w = NKC * S // 4
                nc.sync.dma_start(out=x_sb["q"][:, ch * w:(ch + 1) * w],
                                  in_=x_d["q"][:, ch * w:(ch + 1) * w])
            nc.scalar.dma_start(out=w_sb["k"][:], in_=w_d["k"][:])
            for ch in range(4):
                w = NKC * S // 4
                nc.scalar.dma_start(out=x_sb["k"][:, ch * w:(ch + 1) * w],
                                    in_=x_d["k"][:, ch * w:(ch + 1) * w])
            nc.sync.dma_start(out=w_sb["v"][:], in_=w_d["v"][:])
            for ch in range(4):  # xv is s-major: cols st*1024 + kc*128
                w = NKC * S // 4
                nc.sync.dma_start(out=x_sb["v"][:, ch * w:(ch + 1) * w],
                                  in_=x_d["v"][:, ch * w:(ch + 1) * w])
            band_sb = [bpool.tile([P, NST * 3 * P], BF16, name="band")
                       for h in range(HPC)]
            nc.scalar.dma_start(out=band_sb[0][:], in_=band_d[0])
            nc.scalar.dma_start(out=band_sb[1][:], in_=band_d[1])
            nc.sync.dma_start(out=wo_sb[:], in_=wo_d[:])
            nc.sync.dma_start(out=band_sb[2][:], in_=band_d[2])
            nc.sync.dma_start(out=band_sb[3][:], in_=band_d[3])

            # ---- helpers ----
            def qk_proj(nm, mt, nb, dst):
                ps = ppsum.tile([P, 512], FP32, name="pp")
                for kc in range(NKC):
                    nc.tensor.matmul(
                        ps[:],
                        lhsT=w_sb[nm][:, kc * DHC + mt * P:kc * DHC + mt * P + P],
                        rhs=x_sb[nm][:, kc * S + nb * 512:kc * S + nb * 512 + 512],
                        start=(kc == 0), stop=(kc == NKC - 1))
                nc.vector.tensor_copy(dst[mt][:, nb * 512:nb * 512 + 512], ps[:])

            def klohi(mt, nb):
                sl = slice(nb * 512, nb * 512 + 512)
                nc.vector.tensor_scalar_add(
                    kLo_sb[mt][:, sl], kT_sb[mt][:, sl], pemb32_sb[:])
                nc.vector.tensor_scalar_add(
                    kHi_sb[mt][:, sl], kT_sb[mt][:, sl], pemb0_sb[:])

            def v_proj(st):
                ps = ppsum.tile([P, 512], FP32, name="pp")
                for kc in range(NKC):
                    nc.tensor.matmul(
                        ps[:, 0:DHC],
                        lhsT=x_sb["v"][:, st * (NKC * P) + kc * P:
                                       st * (NKC * P) + kc * P + P],
                        rhs=w_sb["v"][:, kc * DHC:(kc + 1) * DHC],
                        start=(kc == 0), stop=(kc == NKC - 1))
                nc.vector.memset(v_sb[st][:], 1.0)
                for hh in range(HPC):
                    nc.vector.tensor_copy(
                        v_sb[st][:, 65 * hh:65 * hh + ADIM],
                        ps[:, ADIM * hh:ADIM * hh + ADIM])

            def out_proj(tt, nb, eng):
                ps = ppsum.tile([P, 512], FP32, name="pp")
                for cc in range(2):
                    nc.tensor.matmul(
                        ps[:],
                        lhsT=ctxT2_sb[cc][:, tt * P:tt * P + P],
                        rhs=wo_sb[:, cc * D + nb * 512:cc * D + nb * 512 + 512],
                        start=(cc == 0), stop=(cc == 1))
                st_t = ostage.tile([P, 512], BF16, name="ost")
                if eng == 0:
                    nc.vector.tensor_copy(st_t[:], ps[:])
                else:
                    nc.scalar.activation(st_t[:], ps[:],
                                         mybir.ActivationFunctionType.Copy)
                nc.sync.dma_start(
                    out=out_d[tt * P:tt * P + P, nb * 512:nb * 512 + 512],
                    in_=st_t[:])

            # ---- upfront: q/k projections for mt=0 (fed by both queues) ----
            for nb in range(4):
                qk_proj("q", 0, nb, qT_sb)
                qk_proj("k", 0, nb, kT_sb)
                klohi(0, nb)

            # fill-work schedule keyed by iteration: it0 (h0,qh0) runs one
            # v-projection per slot; h1 (it2/3) spreads the mt=1 q/k
            # projections; it7 (h3,qh1) runs the first half of the output
            # projection one (tt, nb) unit per slot.
            f1 = []
            for nb in range(4):
                f1.append(lambda nb=nb: qk_proj("q", 1, nb, qT_sb))
                f1.append(lambda nb=nb: (qk_proj("k", 1, nb, kT_sb),
                                         klohi(1, nb)))
            fills = {
                0: [(lambda st=st: v_proj(st)) for st in range(NST)],
                2: f1, 3: f1,
                7: [(lambda tt=tt, nb=nb: out_proj(tt, nb, (tt + nb) % 2))
                    for tt in range(8) for nb in range(2)],
            }

            ksrc = (kT_sb, kLo_sb, kHi_sb)

            # ---- softmax loop: 8 iterations of (head, q-half) x 16 s-tiles ----
            for it in range(8):
                h, qh = it // 2, it % 2
                mt, po = h // 2, ADIM * (h % 2)
                fq = fills.get(it, [])
                ctx_ps = cpsum.tile([65, QH], FP32, name="ctx")
                prev = None  # (expT, st) pending AV
                for st in range(NST):
                    # scores for this s-tile, q columns [qh*1024, qh*1024+1024)
                    sp = spsum.tile([P, QH], FP32, name="scores")
                    runs = []
                    for tt in range(8 * qh, 8 * qh + 8):
                        dd = st - tt
                        kv = 1 if dd >= 2 else (2 if dd <= -2 else 0)
                        if runs and runs[-1][2] == kv and (tt % 4) != 0:
                            runs[-1][1] = tt + 1
                        else:
                            runs.append([tt, tt + 1, kv])
                    for ta, tb, kv in runs:
                        co = (ta - 8 * qh) * P
                        nc.tensor.matmul(
                            sp[:, co:co + (tb - ta) * P],
                            lhsT=ksrc[kv][mt][po:po + ADIM, st * P:st * P + P],
                            rhs=qT_sb[mt][po:po + ADIM, ta * P:tb * P],
                            start=True, stop=True)
                    expT = epool.tile([P, QH], BF16, name="expT")
                    nc.scalar.activation(expT[:], sp[:],
                                         mybir.ActivationFunctionType.Exp)
                    # multiplicative band on diagonal-crossing tiles in this half
                    pres = [(sl, st - 1 + sl) for sl in range(3)
                            if 0 <= st - 1 + sl < NST
                            and (st - 1 + sl) // 8 == qh]
                    if pres:
                        sl0, tt0 = pres[0]
                        wdt = len(pres) * P
                        lc = (tt0 - 8 * qh) * P
                        bo = (st * 3 + sl0) * P
                        nc.vector.tensor_mul(
                            expT[:, lc:lc + wdt], expT[:, lc:lc + wdt],
                            band_sb[h][:, bo:bo + wdt])
                    # interleaved fill work (projections / out-projection):
                    # it0/it7 pop one unit per slot, h1 one per 4 slots
                    if fq and (it in (0, 7) or st % 4 == 0):
                        fq.pop(0)()
                    # staggered AV (one s-tile behind the scores)
                    if prev is not None:
                        eT, pst = prev
                        for c in range(2):
                            nc.tensor.matmul(
                                ctx_ps[:, c * 512:c * 512 + 512],
                                lhsT=v_sb[pst][:, 65 * h:65 * h + 65],
                                rhs=eT[:, c * 512:c * 512 + 512],
                                start=(pst == 0), stop=(pst == NST - 1))
                    prev = (expT, st)
                eT, pst = prev
                for c in range(2):
                    nc.tensor.matmul(
                        ctx_ps[:, c * 512:c * 512 + 512],
                        lhsT=v_sb[pst][:, 65 * h:65 * h + 65],
                        rhs=eT[:, c * 512:c * 512 + 512],
                        start=(pst == 0), stop=(pst == NST - 1))
                # leftover fill work
                while fq:
                    fq.pop(0)()
                # normalize: recip of denominator row, broadcast via K=1
                # matmul, multiply+cast into the head-pair tile
                with nc.allow_low_precision(reason="bf16 recip of softmax "
                                            "denominator, ~0.3% common-mode"):
                    nc.vector.reciprocal(dn_sb[64:65, :], ctx_ps[64:65, :])
                for c in range(2):
                    bc = ppsum.tile([P, 512], FP32, name="pp")
                    nc.tensor.matmul(
                        bc[0:64, :],
                        lhsT=ones_sb[64:65, 0:64],
                        rhs=dn_sb[64:65, c * 512:c * 512 + 512],
                        start=True, stop=True)
                    csl = slice(c * 512, c * 512 + 512)
                    nc.vector.tensor_copy(bc_sb[0:64, csl], bc[0:64, :])
                    if h % 2 == 0:
                        nc.vector.tensor_mul(
                            ctxT2_sb[mt][0:64, qh * QH + c * 512:
                                         qh * QH + c * 512 + 512],
                            ctx_ps[0:64, csl], bc_sb[0:64, csl])
                    else:
                        nc.vector.tensor_mul(
                            tmp_sb[0:64, csl], ctx_ps[0:64, csl],
                            bc_sb[0:64, csl])
                if h % 2 == 1:
                    nc.sync.dma_start(
                        out=ctxT2_sb[mt][64:128, qh * QH:qh * QH + QH],
                        in_=tmp_sb[0:64, :])

            # ---- tail of the output projection ----
            for tt in range(8, NST):
                for nb in range(2):
                    out_proj(tt, nb, (tt + nb) % 2)
    nc.compile()
    return nc


def _bf16(x):
    return np.ascontiguousarray(np.asarray(x, np.float32)).astype(
        ml_dtypes.bfloat16)


def _swiz(xT):
    """[D, S]-like -> SBUF layout [128, (D/128)*S] (chunk kc at cols kc*S)."""
    d0, s0 = xT.shape
    return np.ascontiguousarray(
        xT.reshape(d0 // P, P, s0).transpose(1, 0, 2).reshape(P, -1))


def _swiz_smajor(xT):
    """[D, S] -> [128, st*1024 + kc*128 + c] (s-tile major for v proj)."""
    d0, s0 = xT.shape
    return np.ascontiguousarray(
        xT.reshape(NKC, P, NST, P).transpose(1, 2, 0, 3).reshape(P, -1))


def _host_inputs(iQ, iK, iV, Wq, Wk, Wv, Wo, rel_pemb):
    iQ, iK, iV = (np.asarray(a, np.float32) for a in (iQ, iK, iV))
    Wq, Wk, Wv, Wo = (np.asarray(a, np.float32) for a in (Wq, Wk, Wv, Wo))
    rel_pemb = np.asarray(rel_pemb, np.float32)
    pembT = rel_pemb.T
    pemb0 = np.tile(rel_pemb[0], 2).reshape(P, 1).astype(np.float32)
    pemb32 = np.tile(rel_pemb[32], 2).reshape(P, 1).astype(np.float32)

    sl = np.arange(P)[:, None]
    tl = np.arange(P)[None, :]
    idx_d = {d: np.clip(d + sl - tl + K_REL, 0, NJ - 1) for d in (128, 0, -128)}
    slot_d = (128, 0, -128)

    in_maps = []
    for c in range(8):
        b, g = c // 4, c % 4
        cols = slice(DHC * g, DHC * g + DHC)
        Qg = (iQ[b] @ Wq[:, cols]) * 0.125
        band = np.zeros((HPC, NST, 3, P, P), np.float32)
        for h in range(HPC):
            ph = Qg[:, ADIM * h:ADIM * h + ADIM] @ pembT
            for st in range(NST):
                for slot, d in enumerate(slot_d):
                    tt = st - 1 + slot
                    if not 0 <= tt < NST:
                        continue
                    pb = ph[tt * P:tt * P + P]
                    band[h, st, slot] = pb[tl, idx_d[d]]
        band = np.exp(band)
        band = np.ascontiguousarray(band.transpose(0, 3, 1, 2, 4)
                                    .reshape(HPC, P, NST * 3 * P))
        in_maps.append({
            "xq": _bf16(_swiz(iQ[b].T)), "xk": _bf16(_swiz(iK[b].T)),
            "xv": _bf16(_swiz_smajor(iV[b].T)),
            "wq": _bf16(_swiz(Wq[:, cols] * 0.125)),
            "wk": _bf16(_swiz(Wk[:, cols])),
            "wv": _bf16(_swiz(Wv[:, cols])), "wo": _bf16(_swiz(Wo[cols, :])),
            "pemb0": pemb0, "pemb32": pemb32, "band": _bf16(band),
        })
    return in_maps


def kernel(iQ, iK, iV, Wq, Wk, Wv, Wo, rel_pemb, _trace=False):
    global _COMPILED
    if _COMPILED is None:
        _COMPILED = build_nc()
    nc = _COMPILED
    in_maps = _host_inputs(iQ, iK, iV, Wq, Wk, Wv, Wo, rel_pemb)
    res = run_bass_kernel_spmd(nc, in_maps, list(range(8)), trace=_trace)
    parts = [res.results[c]["out"].astype(np.float32) for c in range(8)]
    out = np.stack([parts[0] + parts[1] + parts[2] + parts[3],
                    parts[4] + parts[5] + parts[6] + parts[7]])
    if _trace:
        return out, res
    return out


# revision 18
# speedup vs baseline: 1.1978x; 1.1978x over previous
"""Trainium2 Bass kernel for nn_MultiHeadAttn (B=2, S=2048, D=1024, H=16,
ADIM=64, rel-pos bias vocab 33).

Sharding: batch x head-group over 8 cores. Core c handles batch b=c//4 and
heads [4*(c%4), 4*(c%4)+4). Each core computes q/k/v projections for its 256
model dims, attention for its 4 heads, and a partial output projection; the
host sums the 4 partials per batch.

Rel-pos bias (same trick as before): scoresT[s,t] uses k VARIANTS so the far
field is free (kLo = k + pemb[32] for s-t >= 2 tiles, kHi = k + pemb[0] for
t-s >= 2 tiles); the <=3 diagonal-crossing tiles get their bias
multiplicatively after exp via a host-precomputed band.

This version:
  * AV is swapped: v (with a ones column for the denominator) is the
    STATIONARY operand, expT the moving one -> ctxT accumulates directly in
    PSUM as [65, q] (2 matmuls of N=512 per (head, q-half, s-tile) instead
    of 16 matmuls of N=65). No PE transposes needed for the out projection.
  * q is processed in two 1024-col halves per head so ctx PSUM is 2 banks,
    leaving banks for projections to interleave into the softmax loop:
    v-projection fills head 0, the mt=1 q/k projections fill head 1, and
    the first half of the output projection fills head 3's second half.
  * Softmax normalization: reciprocal of the PSUM denominator row, a K=1
    ones-matmul broadcasts it across partitions, one vector multiply
    normalizes and casts; odd heads are packed into partitions 64..127 of
    the pair tile via a small SBUF->SBUF DMA.
  * Partial outputs returned in bf16 (halves the output DMA).
"""
import numpy as np
import ml_dtypes

import concourse.bacc as bacc
import concourse.mybir as mybir
import concourse.tile as tile
from concourse.bass_utils import run_bass_kernel_spmd

B, S, D = 2, 2048, 1024
H, ADIM, K_REL, NJ = 16, 64, 16, 33
HPC = 4            # heads per core
DHC = HPC * ADIM   # 256 model dims per core
P = 128
NST = S // P       # 16 s-tiles
NKC = D // P       # 8 contraction chunks for projections
QH = 1024          # q processed in halves
BF16 = mybir.dt.bfloat16
FP32 = mybir.dt.float32

_COMPILED = None


def build_nc():
    nc = bacc.Bacc(None, target_bir_lowering=False)
    with tile.TileContext(nc) as tc:
        x_d = {nm: nc.dram_tensor(f"x{nm}", [P, NKC * S], BF16,
                                  kind="ExternalInput") for nm in "qkv"}
        w_d = {nm: nc.dram_tensor(f"w{nm}", [P, NKC * DHC], BF16,
                                  kind="ExternalInput") for nm in "qkv"}
        wo_d = nc.dram_tensor("wo", [P, 2 * D], BF16, kind="ExternalInput")
        pemb0_d = nc.dram_tensor("pemb0", [P, 1], FP32, kind="ExternalInput")
        pemb32_d = nc.dram_tensor("pemb32", [P, 1], FP32, kind="ExternalInput")
        band_d = nc.dram_tensor("band", [HPC, P, NST * 3 * P], BF16,
                                kind="ExternalInput")
        out_d = nc.dram_tensor("out", [S, D], BF16, kind="ExternalOutput")

        from contextlib import ExitStack
        with ExitStack() as stack:
            const = stack.enter_context(tc.tile_pool(name="const", bufs=1))
            pemb0_sb = const.tile([P, 1], FP32)
            pemb32_sb = const.tile([P, 1], FP32)
            nc.sync.dma_start(out=pemb0_sb[:], in_=pemb0_d[:])
            nc.sync.dma_start(out=pemb32_sb[:], in_=pemb32_d[:])

            persist = stack.enter_context(tc.tile_pool(name="persist", bufs=1))
            qT_sb = [persist.tile([P, S], BF16, name=f"qT{i}") for i in range(2)]
            kT_sb = [persist.tile([P, S], BF16, name=f"kT{i}") for i in range(2)]
            kLo_sb = [persist.tile([P, S], BF16, name=f"kLo{i}") for i in range(2)]
            kHi_sb = [persist.tile([P, S], BF16, name=f"kHi{i}") for i in range(2)]
            v_sb = [persist.tile([P, HPC * P], BF16, name=f"v{st}")
                    for st in range(NST)]
            ctxT2_sb = [persist.tile([P, S], BF16, name=f"ctxT2{i}")
                        for i in range(2)]
            wo_sb = persist.tile([P, 2 * D], BF16, name="wo")
            tmp_sb = persist.tile([64, QH], BF16, name="tmp")
            rec_sb = persist.tile([P, QH], BF16, name="rec")
            craw_sb = persist.tile([P, QH], BF16, name="craw")

            xin = stack.enter_context(tc.tile_pool(name="xin", bufs=1))
            w_in = stack.enter_context(tc.tile_pool(name="w_in", bufs=1))
            x_sb = {nm: xin.tile([P, NKC * S], BF16, name=f"x{nm}")
                    for nm in "qkv"}
            w_sb = {nm: w_in.tile([P, NKC * DHC], BF16, name=f"w{nm}")
                    for nm in "qkv"}

            ppsum = stack.enter_context(
                tc.tile_pool(name="ppsum", bufs=2, space="PSUM"))
            spsum = stack.enter_context(
                tc.tile_pool(name="spsum", bufs=2, space="PSUM"))
            cpsum = stack.enter_context(
                tc.tile_pool(name="cpsum", bufs=1, space="PSUM"))
            epool = stack.enter_context(tc.tile_pool(name="expT", bufs=3))
            bpool = stack.enter_context(tc.tile_pool(name="band", bufs=2))
            ostage = stack.enter_context(tc.tile_pool(name="ostage", bufs=2))

            # ---- input DMAs: xq + xv on sync queue, xk + bands on scalar
            # queue (both are HWDGE queues; scalar is idle until first exp) ----
            nc.sync.dma_start(out=w_sb["q"][:], in_=w_d["q"][:])
            for ch in range(4):
                w = NKC * S // 4
                nc.sync.dma_start(out=x_sb["q"][:, ch * w:(ch + 1) * w],
                                  in_=x_d["q"][:, ch * w:(ch + 1) * w])
            nc.scalar.dma_start(out=w_sb["k"][:], in_=w_d["k"][:])
            for ch in range(4):
                w = NKC * S // 4
                nc.scalar.dma_start(out=x_sb["k"][:, ch * w:(ch + 1) * w],
                                    in_=x_d["k"][:, ch * w:(ch + 1) * w])
            nc.sync.dma_start(out=w_sb["v"][:], in_=w_d["v"][:])
            for ch in range(4):  # xv is s-major: cols st*1024 + kc*128
                w = NKC * S // 4
                nc.sync.dma_start(out=x_sb["v"][:, ch * w:(ch + 1) * w],
                                  in_=x_d["v"][:, ch * w:(ch + 1) * w])
            band_sb = [bpool.tile([P, NST * 3 * P], BF16, name="band")
                       for h in range(HPC)]
            nc.scalar.dma_start(out=band_sb[0][:], in_=band_d[0])
            nc.scalar.dma_start(out=band_sb[1][:], in_=band_d[1])
            nc.sync.dma_start(out=wo_sb[:], in_=wo_d[:])
            nc.sync.dma_start(out=band_sb[2][:], in_=band_d[2])
            nc.sync.dma_start(out=band_sb[3][:], in_=band_d[3])

            # ---- helpers ----
            def qk_proj(nm, mt, nb, dst):
                ps = ppsum.tile([P, 512], FP32, name="pp")
                for kc in range(NKC):
                    nc.tensor.matmul(
                        ps[:],
                        lhsT=w_sb[nm][:, kc * DHC + mt * P:kc * DHC + mt * P + P],
                        rhs=x_sb[nm][:, kc * S + nb * 512:kc * S + nb * 512 + 512],
                        start=(kc == 0), stop=(kc == NKC - 1))
                nc.vector.tensor_copy(dst[mt][:, nb * 512:nb * 512 + 512], ps[:])

            def klohi(mt, nb):
                sl = slice(nb * 512, nb * 512 + 512)
                nc.vector.tensor_scalar_add(
                    kLo_sb[mt][:, sl], kT_sb[mt][:, sl], pemb32_sb[:])
                nc.vector.tensor_scalar_add(
                    kHi_sb[mt][:, sl], kT_sb[mt][:, sl], pemb0_sb[:])

            def v_proj(st):
                ps = ppsum.tile([P, 512], FP32, name="pp")
                for kc in range(NKC):
                    nc.tensor.matmul(
                        ps[:, 0:DHC],
                        lhsT=x_sb["v"][:, st * (NKC * P) + kc * P:
                                       st * (NKC * P) + kc * P + P],
                        rhs=w_sb["v"][:, kc * DHC:(kc + 1) * DHC],
                        start=(kc == 0), stop=(kc == NKC - 1))
                nc.vector.memset(v_sb[st][:], 1.0)
                for hh in range(HPC):
                    nc.vector.tensor_copy(
                        v_sb[st][:, P * hh:P * hh + ADIM],
                        ps[:, ADIM * hh:ADIM * hh + ADIM])

            def out_proj(tt, nb, eng):
                ps = ppsum.tile([P, 512], FP32, name="pp")
                for cc in range(2):
                    nc.tensor.matmul(
                        ps[:],
                        lhsT=ctxT2_sb[cc][:, tt * P:tt * P + P],
                        rhs=wo_sb[:, cc * D + nb * 512:cc * D + nb * 512 + 512],
                        start=(cc == 0), stop=(cc == 1))
                st_t = ostage.tile([P, 512], BF16, name="ost")
                if eng == 0:
                    nc.vector.tensor_copy(st_t[:], ps[:])
                else:
                    nc.scalar.activation(st_t[:], ps[:],
                                         mybir.ActivationFunctionType.Copy)
                nc.sync.dma_start(
                    out=out_d[tt * P:tt * P + P, nb * 512:nb * 512 + 512],
                    in_=st_t[:])

            # ---- upfront: q/k projections for mt=0 (fed by both queues) ----
            for nb in range(4):
                qk_proj("q", 0, nb, qT_sb)
                qk_proj("k", 0, nb, kT_sb)
                klohi(0, nb)

            # fill-work schedule keyed by iteration: it0 (h0,qh0) runs one
            # v-projection per slot; h1 (it2/3) spreads the mt=1 q/k
            # projections; it7 (h3,qh1) runs the first half of the output
            # projection one (tt, nb) unit per slot.
            f1 = []
            for nb in range(4):
                f1.append(lambda nb=nb: qk_proj("q", 1, nb, qT_sb))
                f1.append(lambda nb=nb: (qk_proj("k", 1, nb, kT_sb),
                                         klohi(1, nb)))
            fills = {
                0: [(lambda st=st: v_proj(st)) for st in range(NST)],
                2: f1, 3: f1,
                7: [(lambda tt=tt, nb=nb: out_proj(tt, nb, (tt + nb) % 2))
                    for tt in range(8) for nb in range(2)],
            }

            ksrc = (kT_sb, kLo_sb, kHi_sb)

            def emit_norm(nh, nqh, nmt):
                """Lazy normalize of the PREVIOUS iteration's ctx, staged in
                craw_sb (rows 0:64 data, 64:128 replicated denominator):
                exact reciprocal (64 lanes), shift-DMA, multiply. Runs with a
                full iteration of slack -- nothing on the PE path waits."""
                with nc.allow_low_precision(reason="bf16 softmax denom recip"):
                    nc.vector.reciprocal(rec_sb[64:128, :], craw_sb[64:128, :])
                nc.sync.dma_start(out=rec_sb[0:64, :], in_=rec_sb[64:128, :])
                for c in range(2):
                    csl = slice(c * 512, c * 512 + 512)
                    if nh % 2 == 0:
                        nc.vector.tensor_mul(
                            ctxT2_sb[nmt][0:64, nqh * QH + c * 512:
                                          nqh * QH + c * 512 + 512],
                            craw_sb[0:64, csl], rec_sb[0:64, csl])
                    else:
                        nc.vector.tensor_mul(
                            tmp_sb[0:64, csl], craw_sb[0:64, csl],
                            rec_sb[0:64, csl])
                if nh % 2 == 1:
                    nc.sync.dma_start(
                        out=ctxT2_sb[nmt][64:128, nqh * QH:nqh * QH + QH],
                        in_=tmp_sb[0:64, :])

            # ---- softmax loop: 8 iterations of (head, q-half) x 16 s-tiles ----
            pending_norm = None
            for it in range(8):
                h, qh = it // 2, it % 2
                mt, po = h // 2, ADIM * (h % 2)
                fq = fills.get(it, [])
                ctx_ps = cpsum.tile([P, QH], FP32, name="ctx")
                pend = []  # (expT, st) pending AV, lag 2
                for st in range(NST):
                    # scores for this s-tile, q columns [qh*1024, qh*1024+1024)
                    sp = spsum.tile([P, QH], FP32, name="scores")
                    runs = []
                    for tt in range(8 * qh, 8 * qh + 8):
                        dd = st - tt
                        kv = 1 if dd >= 2 else (2 if dd <= -2 else 0)
                        if runs and runs[-1][2] == kv and (tt % 4) != 0:
                            runs[-1][1] = tt + 1
                        else:
                            runs.append([tt, tt + 1, kv])
                    for ta, tb, kv in runs:
                        co = (ta - 8 * qh) * P
                        nc.tensor.matmul(
                            sp[:, co:co + (tb - ta) * P],
                            lhsT=ksrc[kv][mt][po:po + ADIM, st * P:st * P + P],
                            rhs=qT_sb[mt][po:po + ADIM, ta * P:tb * P],
                            start=True, stop=True)
                    expT = epool.tile([P, QH], BF16, name="expT")
                    nc.scalar.activation(expT[:], sp[:],
                                         mybir.ActivationFunctionType.Exp)
                    # multiplicative band on diagonal-crossing tiles in this half
                    pres = [(sl, st - 1 + sl) for sl in range(3)
                            if 0 <= st - 1 + sl < NST
                            and (st - 1 + sl) // 8 == qh]
                    if pres:
                        sl0, tt0 = pres[0]
                        wdt = len(pres) * P
                        lc = (tt0 - 8 * qh) * P
                        bo = (st * 3 + sl0) * P
                        nc.vector.tensor_mul(
                            expT[:, lc:lc + wdt], expT[:, lc:lc + wdt],
                            band_sb[h][:, bo:bo + wdt])
                    # previous iteration's lazy normalization slots in
                    # here (reads only the craw staging tile)
                    if st == 3 and pending_norm is not None:
                        emit_norm(*pending_norm)
                        pending_norm = None
                    # interleaved fill work (projections / out-projection):
                    # it0 pops one unit per slot, h1 one per 4 slots, it7 one
                    # per slot starting at st=4 (after the it6 norm is emitted)
                    if fq and (it == 0 or (it == 7 and st >= 4)
                               or (it in (2, 3) and st % 4 == 0)):
                        fq.pop(0)()
                        if it == 7 and st >= 12 and fq:
                            fq.pop(0)()
                    # staggered AV (two s-tiles behind the scores)
                    pend.append((expT, st))
                    if len(pend) > 2:
                        eT, pst = pend.pop(0)
                        for c in range(2):
                            nc.tensor.matmul(
                                ctx_ps[:, c * 512:c * 512 + 512],
                                lhsT=v_sb[pst][:, P * h:P * h + P],
                                rhs=eT[:, c * 512:c * 512 + 512],
                                start=(pst == 0), stop=(pst == NST - 1))
                for eT, pst in pend:
                    for c in range(2):
                        nc.tensor.matmul(
                            ctx_ps[:, c * 512:c * 512 + 512],
                            lhsT=v_sb[pst][:, P * h:P * h + P],
                            rhs=eT[:, c * 512:c * 512 + 512],
                            start=(pst == 0), stop=(pst == NST - 1))
                # leftover fill work
                while fq:
                    fq.pop(0)()
                # stage the raw ctx + replicated denominator to SBUF in
                # one fast copy so the ctx PSUM frees immediately; the
                # reciprocal + normalize run lazily next iteration
                nc.vector.tensor_copy(craw_sb[:], ctx_ps[:])
                pending_norm = (h, qh, mt)
            emit_norm(*pending_norm)

            # ---- tail of the output projection ----
            for tt in range(8, NST):
                for nb in range(2):
                    out_proj(tt, nb, (tt + nb) % 2)
    nc.compile()
    return nc


def _bf16(x):
    return np.ascontiguousarray(np.asarray(x, np.float32)).astype(
        ml_dtypes.bfloat16)


def _swiz(xT):
    """[D, S]-like -> SBUF layout [128, (D/128)*S] (chunk kc at cols kc*S)."""
    d0, s0 = xT.shape
    return np.ascontiguousarray(
        xT.reshape(d0 // P, P, s0).transpose(1, 0, 2).reshape(P, -1))


def _swiz_smajor(xT):
    """[D, S] -> [128, st*1024 + kc*128 + c] (s-tile major for v proj)."""
    d0, s0 = xT.shape
    return np.ascontiguousarray(
        xT.reshape(NKC, P, NST, P).transpose(1, 2, 0, 3).reshape(P, -1))


def _host_inputs(iQ, iK, iV, Wq, Wk, Wv, Wo, rel_pemb):
    iQ, iK, iV = (np.asarray(a, np.float32) for a in (iQ, iK, iV))
    Wq, Wk, Wv, Wo = (np.asarray(a, np.float32) for a in (Wq, Wk, Wv, Wo))
    rel_pemb = np.asarray(rel_pemb, np.float32)
    pembT = rel_pemb.T
    pemb0 = np.tile(rel_pemb[0], 2).reshape(P, 1).astype(np.float32)
    pemb32 = np.tile(rel_pemb[32], 2).reshape(P, 1).astype(np.float32)

    sl = np.arange(P)[:, None]
    tl = np.arange(P)[None, :]
    idx_d = {d: np.clip(d + sl - tl + K_REL, 0, NJ - 1) for d in (128, 0, -128)}
    slot_d = (128, 0, -128)

    in_maps = []
    for c in range(8):
        b, g = c // 4, c % 4
        cols = slice(DHC * g, DHC * g + DHC)
        Qg = (iQ[b] @ Wq[:, cols]) * 0.125
        band = np.zeros((HPC, NST, 3, P, P), np.float32)
        for h in range(HPC):
            ph = Qg[:, ADIM * h:ADIM * h + ADIM] @ pembT
            for st in range(NST):
                for slot, d in enumerate(slot_d):
                    tt = st - 1 + slot
                    if not 0 <= tt < NST:
                        continue
                    pb = ph[tt * P:tt * P + P]
                    band[h, st, slot] = pb[tl, idx_d[d]]
        band = np.exp(band)
        band = np.ascontiguousarray(band.transpose(0, 3, 1, 2, 4)
                                    .reshape(HPC, P, NST * 3 * P))
        in_maps.append({
            "xq": _bf16(_swiz(iQ[b].T)), "xk": _bf16(_swiz(iK[b].T)),
            "xv": _bf16(_swiz_smajor(iV[b].T)),
            "wq": _bf16(_swiz(Wq[:, cols] * 0.125)),
            "wk": _bf16(_swiz(Wk[:, cols])),
            "wv": _bf16(_swiz(Wv[:, cols])), "wo": _bf16(_swiz(Wo[cols, :])),
            "pemb0": pemb0, "pemb32": pemb32, "band": _bf16(band),
        })
    return in_maps


def kernel(iQ, iK, iV, Wq, Wk, Wv, Wo, rel_pemb, _trace=False):
    global _COMPILED
    if _COMPILED is None:
        _COMPILED = build_nc()
    nc = _COMPILED
    in_maps = _host_inputs(iQ, iK, iV, Wq, Wk, Wv, Wo, rel_pemb)
    res = run_bass_kernel_spmd(nc, in_maps, list(range(8)), trace=_trace)
    parts = [res.results[c]["out"].astype(np.float32) for c in range(8)]
    out = np.stack([parts[0] + parts[1] + parts[2] + parts[3],
                    parts[4] + parts[5] + parts[6] + parts[7]])
    if _trace:
        return out, res
    return out


# revision 19
# speedup vs baseline: 1.2163x; 1.0154x over previous
"""Trainium2 Bass kernel for nn_MultiHeadAttn (B=2, S=2048, D=1024, H=16,
ADIM=64, rel-pos bias vocab 33).

Sharding: batch x head-group over 8 cores. Core c handles batch b=c//4 and
heads [4*(c%4), 4*(c%4)+4). Each core computes q/k/v projections for its 256
model dims, attention for its 4 heads, and a partial output projection; the
host sums the 4 partials per batch.

Rel-pos bias (same trick as before): scoresT[s,t] uses k VARIANTS so the far
field is free (kLo = k + pemb[32] for s-t >= 2 tiles, kHi = k + pemb[0] for
t-s >= 2 tiles); the <=3 diagonal-crossing tiles get their bias
multiplicatively after exp via a host-precomputed band.

This version:
  * AV is swapped: v (with a ones column for the denominator) is the
    STATIONARY operand, expT the moving one -> ctxT accumulates directly in
    PSUM as [65, q] (2 matmuls of N=512 per (head, q-half, s-tile) instead
    of 16 matmuls of N=65). No PE transposes needed for the out projection.
  * q is processed in two 1024-col halves per head so ctx PSUM is 2 banks,
    leaving banks for projections to interleave into the softmax loop:
    v-projection fills head 0, the mt=1 q/k projections fill head 1, and
    the first half of the output projection fills head 3's second half.
  * Softmax normalization: reciprocal of the PSUM denominator row, a K=1
    ones-matmul broadcasts it across partitions, one vector multiply
    normalizes and casts; odd heads are packed into partitions 64..127 of
    the pair tile via a small SBUF->SBUF DMA.
  * Partial outputs returned in bf16 (halves the output DMA).
"""
import numpy as np
import ml_dtypes

import concourse.bacc as bacc
import concourse.mybir as mybir
import concourse.tile as tile
from concourse.bass_utils import run_bass_kernel_spmd

B, S, D = 2, 2048, 1024
H, ADIM, K_REL, NJ = 16, 64, 16, 33
HPC = 4            # heads per core
DHC = HPC * ADIM   # 256 model dims per core
P = 128
NST = S // P       # 16 s-tiles
NKC = D // P       # 8 contraction chunks for projections
QH = 1024          # q processed in halves
BF16 = mybir.dt.bfloat16
FP32 = mybir.dt.float32

_COMPILED = None


def build_nc():
    nc = bacc.Bacc(None, target_bir_lowering=False)
    with tile.TileContext(nc) as tc:
        x_d = {nm: nc.dram_tensor(f"x{nm}", [P, NKC * S], BF16,
                                  kind="ExternalInput") for nm in "qkv"}
        w_d = {nm: nc.dram_tensor(f"w{nm}", [P, NKC * DHC], BF16,
                                  kind="ExternalInput") for nm in "qkv"}
        wo_d = nc.dram_tensor("wo", [P, 2 * D], BF16, kind="ExternalInput")
        pemb0_d = nc.dram_tensor("pemb0", [P, 1], FP32, kind="ExternalInput")
        pemb32_d = nc.dram_tensor("pemb32", [P, 1], FP32, kind="ExternalInput")
        band_d = nc.dram_tensor("band", [HPC, P, NST * 3 * P], BF16,
                                kind="ExternalInput")
        out_d = nc.dram_tensor("out", [S, D], BF16, kind="ExternalOutput")

        from contextlib import ExitStack
        with ExitStack() as stack:
            const = stack.enter_context(tc.tile_pool(name="const", bufs=1))
            pemb0_sb = const.tile([P, 1], FP32)
            pemb32_sb = const.tile([P, 1], FP32)
            nc.sync.dma_start(out=pemb0_sb[:], in_=pemb0_d[:])
            nc.sync.dma_start(out=pemb32_sb[:], in_=pemb32_d[:])

            persist = stack.enter_context(tc.tile_pool(name="persist", bufs=1))
            qT_sb = [persist.tile([P, S], BF16, name=f"qT{i}") for i in range(2)]
            kT_sb = [persist.tile([P, S], BF16, name=f"kT{i}") for i in range(2)]
            kLo_sb = [persist.tile([P, S], BF16, name=f"kLo{i}") for i in range(2)]
            kHi_sb = [persist.tile([P, S], BF16, name=f"kHi{i}") for i in range(2)]
            v_sb = [persist.tile([P, HPC * P], BF16, name=f"v{st}")
                    for st in range(NST)]
            ctxT2_sb = [persist.tile([P, S], BF16, name=f"ctxT2{i}")
                        for i in range(2)]
            wo_sb = persist.tile([P, 2 * D], BF16, name="wo")
            tmp_sb = persist.tile([64, QH], BF16, name="tmp")
            rec_sb = persist.tile([P, QH], BF16, name="rec")
            craw_sb = persist.tile([P, QH], BF16, name="craw")

            xin = stack.enter_context(tc.tile_pool(name="xin", bufs=1))
            w_in = stack.enter_context(tc.tile_pool(name="w_in", bufs=1))
            x_sb = {nm: xin.tile([P, NKC * S], BF16, name=f"x{nm}")
                    for nm in "qkv"}
            w_sb = {nm: w_in.tile([P, NKC * DHC], BF16, name=f"w{nm}")
                    for nm in "qkv"}

            ppsum = stack.enter_context(
                tc.tile_pool(name="ppsum", bufs=2, space="PSUM"))
            spsum = stack.enter_context(
                tc.tile_pool(name="spsum", bufs=2, space="PSUM"))
            cpsum = stack.enter_context(
                tc.tile_pool(name="cpsum", bufs=1, space="PSUM"))
            epool = stack.enter_context(tc.tile_pool(name="expT", bufs=3))
            bpool = stack.enter_context(tc.tile_pool(name="band", bufs=2))
            ostage = stack.enter_context(tc.tile_pool(name="ostage", bufs=2))

            # ---- input DMAs: xq + xv on sync queue, xk + bands on scalar
            # queue (both are HWDGE queues; scalar is idle until first exp) ----
            nc.sync.dma_start(out=w_sb["q"][:], in_=w_d["q"][:])
            for ch in range(4):
                w = NKC * S // 4
                nc.sync.dma_start(out=x_sb["q"][:, ch * w:(ch + 1) * w],
                                  in_=x_d["q"][:, ch * w:(ch + 1) * w])
            nc.scalar.dma_start(out=w_sb["k"][:], in_=w_d["k"][:])
            for ch in range(4):
                w = NKC * S // 4
                nc.scalar.dma_start(out=x_sb["k"][:, ch * w:(ch + 1) * w],
                                    in_=x_d["k"][:, ch * w:(ch + 1) * w])
            nc.sync.dma_start(out=w_sb["v"][:], in_=w_d["v"][:])
            for ch in range(4):  # xv is s-major: cols st*1024 + kc*128
                w = NKC * S // 4
                nc.sync.dma_start(out=x_sb["v"][:, ch * w:(ch + 1) * w],
                                  in_=x_d["v"][:, ch * w:(ch + 1) * w])
            band_sb = [bpool.tile([P, NST * 3 * P], BF16, name="band")
                       for h in range(HPC)]
            nc.scalar.dma_start(out=band_sb[0][:], in_=band_d[0])
            nc.scalar.dma_start(out=band_sb[1][:], in_=band_d[1])
            nc.sync.dma_start(out=wo_sb[:], in_=wo_d[:])
            nc.sync.dma_start(out=band_sb[2][:], in_=band_d[2])
            nc.sync.dma_start(out=band_sb[3][:], in_=band_d[3])

            # ---- helpers ----
            def qk_proj(nm, mt, nb, dst):
                ps = ppsum.tile([P, 512], FP32, name="pp")
                for kc in range(NKC):
                    nc.tensor.matmul(
                        ps[:],
                        lhsT=w_sb[nm][:, kc * DHC + mt * P:kc * DHC + mt * P + P],
                        rhs=x_sb[nm][:, kc * S + nb * 512:kc * S + nb * 512 + 512],
                        start=(kc == 0), stop=(kc == NKC - 1))
                nc.vector.tensor_copy(dst[mt][:, nb * 512:nb * 512 + 512], ps[:])

            def klohi(mt, nb):
                sl = slice(nb * 512, nb * 512 + 512)
                nc.vector.tensor_scalar_add(
                    kLo_sb[mt][:, sl], kT_sb[mt][:, sl], pemb32_sb[:])
                nc.vector.tensor_scalar_add(
                    kHi_sb[mt][:, sl], kT_sb[mt][:, sl], pemb0_sb[:])

            def v_proj(st):
                ps = ppsum.tile([P, 512], FP32, name="pp")
                for kc in range(NKC):
                    nc.tensor.matmul(
                        ps[:, 0:DHC],
                        lhsT=x_sb["v"][:, st * (NKC * P) + kc * P:
                                       st * (NKC * P) + kc * P + P],
                        rhs=w_sb["v"][:, kc * DHC:(kc + 1) * DHC],
                        start=(kc == 0), stop=(kc == NKC - 1))
                nc.vector.memset(v_sb[st][:], 1.0)
                for hh in range(HPC):
                    nc.vector.tensor_copy(
                        v_sb[st][:, P * hh:P * hh + ADIM],
                        ps[:, ADIM * hh:ADIM * hh + ADIM])

            def out_proj(tt, nb, eng):
                ps = ppsum.tile([P, 512], FP32, name="pp")
                for cc in range(2):
                    nc.tensor.matmul(
                        ps[:],
                        lhsT=ctxT2_sb[cc][:, tt * P:tt * P + P],
                        rhs=wo_sb[:, cc * D + nb * 512:cc * D + nb * 512 + 512],
                        start=(cc == 0), stop=(cc == 1))
                st_t = ostage.tile([P, 512], BF16, name="ost")
                if eng == 0:
                    nc.vector.tensor_copy(st_t[:], ps[:])
                else:
                    nc.scalar.activation(st_t[:], ps[:],
                                         mybir.ActivationFunctionType.Copy)
                nc.sync.dma_start(
                    out=out_d[tt * P:tt * P + P, nb * 512:nb * 512 + 512],
                    in_=st_t[:])

            # ---- upfront: q/k projections for mt=0 (fed by both queues) ----
            for nb in range(4):
                qk_proj("q", 0, nb, qT_sb)
                qk_proj("k", 0, nb, kT_sb)
                klohi(0, nb)

            # fill-work schedule keyed by iteration: it0 (h0,qh0) runs one
            # v-projection per slot; h1 (it2/3) spreads the mt=1 q/k
            # projections; it7 (h3,qh1) runs the first half of the output
            # projection one (tt, nb) unit per slot.
            f1 = []
            for nb in range(4):
                f1.append(lambda nb=nb: qk_proj("q", 1, nb, qT_sb))
                f1.append(lambda nb=nb: (qk_proj("k", 1, nb, kT_sb),
                                         klohi(1, nb)))
            fills = {
                0: [(lambda st=st: v_proj(st)) for st in range(NST)],
                2: f1, 3: f1,
                7: [(lambda tt=tt, nb=nb: out_proj(tt, nb, (tt + nb) % 2))
                    for tt in range(8) for nb in range(2)],
            }

            ksrc = (kT_sb, kLo_sb, kHi_sb)

            def norm_piece(pn, j):
                """Piece j (0..8) of the lazy normalization of the previous
                iteration's ctx staged in craw_sb. The reciprocal is split
                into 8 [64,128] chunks so it never blocks the vector FIFO;
                each 512-half gets its shift-DMA + multiply once its chunks
                are done (j==4 covers half 0, j==8 half 1)."""
                nh, nqh, nmt = pn
                if j < 8:
                    cs = slice(j * 128, j * 128 + 128)
                    with nc.allow_low_precision(reason="bf16 denom recip"):
                        nc.vector.reciprocal(rec_sb[64:128, cs],
                                             craw_sb[64:128, cs])
                if j in (4, 8):
                    c = 0 if j == 4 else 1
                    csl = slice(c * 512, c * 512 + 512)
                    nc.sync.dma_start(out=rec_sb[0:64, csl],
                                      in_=rec_sb[64:128, csl])
                    if nh % 2 == 0:
                        nc.vector.tensor_mul(
                            ctxT2_sb[nmt][0:64, nqh * QH + c * 512:
                                          nqh * QH + c * 512 + 512],
                            craw_sb[0:64, csl], rec_sb[0:64, csl])
                    else:
                        nc.vector.tensor_mul(
                            tmp_sb[0:64, csl], craw_sb[0:64, csl],
                            rec_sb[0:64, csl])
                        if c == 1:
                            nc.sync.dma_start(
                                out=ctxT2_sb[nmt][64:128,
                                                  nqh * QH:nqh * QH + QH],
                                in_=tmp_sb[0:64, :])

            def emit_norm(pn):
                for j in range(9):
                    norm_piece(pn, j)

            # ---- softmax loop: 8 iterations of (head, q-half) x 16 s-tiles ----
            pending_norm = None
            for it in range(8):
                h, qh = it // 2, it % 2
                mt, po = h // 2, ADIM * (h % 2)
                fq = fills.get(it, [])
                ctx_ps = cpsum.tile([P, QH], FP32, name="ctx")
                pend = []  # (expT, st) pending AV, lag 2
                for st in range(NST):
                    # scores for this s-tile, q columns [qh*1024, qh*1024+1024)
                    sp = spsum.tile([P, QH], FP32, name="scores")
                    runs = []
                    for tt in range(8 * qh, 8 * qh + 8):
                        dd = st - tt
                        kv = 1 if dd >= 2 else (2 if dd <= -2 else 0)
                        if runs and runs[-1][2] == kv and (tt % 4) != 0:
                            runs[-1][1] = tt + 1
                        else:
                            runs.append([tt, tt + 1, kv])
                    for ta, tb, kv in runs:
                        co = (ta - 8 * qh) * P
                        nc.tensor.matmul(
                            sp[:, co:co + (tb - ta) * P],
                            lhsT=ksrc[kv][mt][po:po + ADIM, st * P:st * P + P],
                            rhs=qT_sb[mt][po:po + ADIM, ta * P:tb * P],
                            start=True, stop=True)
                    expT = epool.tile([P, QH], BF16, name="expT")
                    nc.scalar.activation(expT[:], sp[:],
                                         mybir.ActivationFunctionType.Exp)
                    # multiplicative band on diagonal-crossing tiles in this half
                    pres = [(sl, st - 1 + sl) for sl in range(3)
                            if 0 <= st - 1 + sl < NST
                            and (st - 1 + sl) // 8 == qh]
                    if pres:
                        sl0, tt0 = pres[0]
                        wdt = len(pres) * P
                        lc = (tt0 - 8 * qh) * P
                        bo = (st * 3 + sl0) * P
                        nc.vector.tensor_mul(
                            expT[:, lc:lc + wdt], expT[:, lc:lc + wdt],
                            band_sb[h][:, bo:bo + wdt])
                    # previous iteration's lazy normalization, spread
                    # one small piece per slot (reads only craw staging)
                    if pending_norm is not None and st <= 8:
                        norm_piece(pending_norm, st)
                        if st == 8:
                            pending_norm = None
                    # interleaved fill work (projections / out-projection):
                    # it0 pops one unit per slot, h1 one per 4 slots, it7 two
                    # per slot starting at st=10 (after the it6 norm is done)
                    if fq and (it == 0 or (it == 7 and st >= 10)
                               or (it in (2, 3) and st % 4 == 0)):
                        fq.pop(0)()
                        if it == 7 and fq:
                            fq.pop(0)()
                    # staggered AV (two s-tiles behind the scores)
                    pend.append((expT, st))
                    if len(pend) > 2:
                        eT, pst = pend.pop(0)
                        for c in range(2):
                            nc.tensor.matmul(
                                ctx_ps[:, c * 512:c * 512 + 512],
                                lhsT=v_sb[pst][:, P * h:P * h + P],
                                rhs=eT[:, c * 512:c * 512 + 512],
                                start=(pst == 0), stop=(pst == NST - 1))
                for eT, pst in pend:
                    for c in range(2):
                        nc.tensor.matmul(
                            ctx_ps[:, c * 512:c * 512 + 512],
                            lhsT=v_sb[pst][:, P * h:P * h + P],
                            rhs=eT[:, c * 512:c * 512 + 512],
                            start=(pst == 0), stop=(pst == NST - 1))
                # leftover fill work
                while fq:
                    fq.pop(0)()
                # stage the raw ctx + replicated denominator to SBUF in
                # one fast copy so the ctx PSUM frees immediately; the
                # reciprocal + normalize run lazily next iteration
                nc.vector.tensor_copy(craw_sb[:], ctx_ps[:])
                pending_norm = (h, qh, mt)
            emit_norm(pending_norm)

            # ---- tail of the output projection ----
            for tt in range(8, NST):
                for nb in range(2):
                    out_proj(tt, nb, (tt + nb) % 2)
    nc.compile()
    return nc


def _bf16(x):
    return np.ascontiguousarray(np.asarray(x, np.float32)).astype(
        ml_dtypes.bfloat16)


def _swiz(xT):
    """[D, S]-like -> SBUF layout [128, (D/128)*S] (chunk kc at cols kc*S)."""
    d0, s0 = xT.shape
    return np.ascontiguousarray(
        xT.reshape(d0 // P, P, s0).transpose(1, 0, 2).reshape(P, -1))


def _swiz_smajor(xT):
    """[D, S] -> [128, st*1024 + kc*128 + c] (s-tile major for v proj)."""
    d0, s0 = xT.shape
    return np.ascontiguousarray(
        xT.reshape(NKC, P, NST, P).transpose(1, 2, 0, 3).reshape(P, -1))


def _host_inputs(iQ, iK, iV, Wq, Wk, Wv, Wo, rel_pemb):
    iQ, iK, iV = (np.asarray(a, np.float32) for a in (iQ, iK, iV))
    Wq, Wk, Wv, Wo = (np.asarray(a, np.float32) for a in (Wq, Wk, Wv, Wo))
    rel_pemb = np.asarray(rel_pemb, np.float32)
    pembT = rel_pemb.T
    pemb0 = np.tile(rel_pemb[0], 2).reshape(P, 1).astype(np.float32)
    pemb32 = np.tile(rel_pemb[32], 2).reshape(P, 1).astype(np.float32)

    sl = np.arange(P)[:, None]
    tl = np.arange(P)[None, :]
    idx_d = {d: np.clip(d + sl - tl + K_REL, 0, NJ - 1) for d in (128, 0, -128)}
    slot_d = (128, 0, -128)

    in_maps = []
    for c in range(8):
        b, g = c // 4, c % 4
        cols = slice(DHC * g, DHC * g + DHC)
        Qg = (iQ[b] @ Wq[:, cols]) * 0.125
        band = np.zeros((HPC, NST, 3, P, P), np.float32)
        for h in range(HPC):
            ph = Qg[:, ADIM * h:ADIM * h + ADIM] @ pembT
            for st in range(NST):
                for slot, d in enumerate(slot_d):
                    tt = st - 1 + slot
                    if not 0 <= tt < NST:
                        continue
                    pb = ph[tt * P:tt * P + P]
                    band[h, st, slot] = pb[tl, idx_d[d]]
        band = np.exp(band)
        band = np.ascontiguousarray(band.transpose(0, 3, 1, 2, 4)
                                    .reshape(HPC, P, NST * 3 * P))
        in_maps.append({
            "xq": _bf16(_swiz(iQ[b].T)), "xk": _bf16(_swiz(iK[b].T)),
            "xv": _bf16(_swiz_smajor(iV[b].T)),
            "wq": _bf16(_swiz(Wq[:, cols] * 0.125)),
            "wk": _bf16(_swiz(Wk[:, cols])),
            "wv": _bf16(_swiz(Wv[:, cols])), "wo": _bf16(_swiz(Wo[cols, :])),
            "pemb0": pemb0, "pemb32": pemb32, "band": _bf16(band),
        })
    return in_maps


def kernel(iQ, iK, iV, Wq, Wk, Wv, Wo, rel_pemb, _trace=False):
    global _COMPILED
    if _COMPILED is None:
        _COMPILED = build_nc()
    nc = _COMPILED
    in_maps = _host_inputs(iQ, iK, iV, Wq, Wk, Wv, Wo, rel_pemb)
    res = run_bass_kernel_spmd(nc, in_maps, list(range(8)), trace=_trace)
    parts = [res.results[c]["out"].astype(np.float32) for c in range(8)]
    out = np.stack([parts[0] + parts[1] + parts[2] + parts[3],
                    parts[4] + parts[5] + parts[6] + parts[7]])
    if _trace:
        return out, res
    return out
